# revision 6
# baseline (speedup 1.0000x reference)
"""Trainium2 Bass kernel for nn_SimplicialAttentionLayer2 (sparse_attention).

Math (per attention a with keys/values K_a, mask m_a, queries H):
    s  = (H/16) @ K_a.T                       # scores
    e1 = exp(s);  S1 = sum_k e1;  p1 = e1/S1  # first softmax (shift-free: safe range)
    arg = p1 * m_a                            # mask multiply
    E2 = exp(arg);  out_a = (E2 @ V) / sum_k E2
Final: (out_1 + out_2 + out_3) / 3.

Kernel strategy (8 cores, shard q-rows of H; replicate K/V):
  - "Layout B": score tiles stored [k (partitions), q (free)] so the PV matmul
    uses e2 tiles directly as lhsT (contraction k on partitions) - no transposes
    on chip.  K.T, mask transposes and all layout prep happen on HOST (free).
  - Deferred normalization: E2 = 1 + u; out = (colsum(V) + u@V) / (K_a + sum u).
    colsum(V) is exact fp32 from host, so bf16 u/V only perturb the small
    correction term.
  - S1 via PE ones-column matmul (partition-axis reduce).
  - 1/S1 broadcast over partitions via rank-1 matmul.
  - attn1/2: exact exp on ScalarE (windowed) then u = E2-1.
    attn3 (p1 <= ~0.04): u = arg + arg^2/2 on VectorE (poly error < 1e-5 rel).
"""

import sys

sys.path.insert(0, "/opt/trn_rl_repo")

from contextlib import ExitStack

import numpy as np
import ml_dtypes

import concourse.bass as bass
import concourse.mybir as mybir
from concourse import bacc
from concourse.tile import TileContext
from concourse.bass_utils import run_bass_kernel_spmd

BF16 = mybir.dt.bfloat16
F32 = mybir.dt.float32
NPBF = ml_dtypes.bfloat16
ALU = mybir.AluOpType
AFT = mybir.ActivationFunctionType

P = 128  # partitions
D = 256  # embedding dim
N_CORES = 8


def build_nc(Q, KS, QC=512, WIN=4, PIECE=8192, poly_attns=(2,)):
    """Build the SPMD Bass program for one core.

    Q: query rows per core.  KS: (K1, K2, K3) key counts per attention.
    QC: q-chunk width.  WIN: k-tiles per strip window.  PIECE: max resident
    kt piece length.  poly_attns: attention indices using the poly-exp path.
    """
    KTOT = sum(KS)
    n_chunks = Q // QC
    offs = [0, KS[0], KS[0] + KS[1]]

    nc = bacc.Bacc("TRN2", debug=False, num_devices=N_CORES)
    hqt_d = nc.declare_dram_parameter("hqt", [2, P, Q], BF16, isOutput=False)
    kt_d = nc.declare_dram_parameter("kt", [2, P, KTOT], BF16, isOutput=False)
    mt_d = nc.declare_dram_parameter("mt", [KTOT, Q], BF16, isOutput=False)
    vp_d = nc.declare_dram_parameter("vp", [KTOT, D + 1], BF16, isOutput=False)
    csv_d = nc.declare_dram_parameter("csv", [3, P, D], F32, isOutput=False)
    out_d = nc.declare_dram_parameter("out", [Q, D], F32, isOutput=True)

    n_acc = Q // P

    with TileContext(nc) as tc, ExitStack() as ctx:
        singles = ctx.enter_context(tc.tile_pool(name="singles", bufs=1))
        strip_p = ctx.enter_context(
            tc.tile_pool(name="strip", bufs=max(KS) // P // WIN)
        )
        kt_p = ctx.enter_context(tc.tile_pool(name="ktp", bufs=2))
        mwin_p = ctx.enter_context(tc.tile_pool(name="mwin", bufs=3))
        vt_p = ctx.enter_context(tc.tile_pool(name="vt", bufs=4))
        small_p = ctx.enter_context(tc.tile_pool(name="small", bufs=2))
        ps_s = ctx.enter_context(tc.tile_pool(name="ps_s", bufs=2, space="PSUM"))
        ps_m = ctx.enter_context(tc.tile_pool(name="ps_m", bufs=1, space="PSUM"))
        ps_o = ctx.enter_context(tc.tile_pool(name="ps_o", bufs=4, space="PSUM"))

        # one-time loads / constants
        hqt = [singles.tile([P, Q], BF16, tag=f"hqt{d}", name=f"hqt{d}") for d in range(2)]
        for d in range(2):
            nc.sync.dma_start(hqt[d][:], hqt_d[d])
        csv = [singles.tile([P, D], F32, tag=f"csv{a}", name=f"csv{a}") for a in range(3)]
        for a in range(3):
            nc.sync.dma_start(csv[a][:], csv_d[a])
        ones_k = singles.tile([P, 1], BF16, tag="ones_k", name="ones_k")
        nc.vector.memset(ones_k[:], 1.0)
        ones_r = singles.tile([1, P], F32, tag="ones_r", name="ones_r")
        nc.vector.memset(ones_r[:], 1.0)
        acc = [singles.tile([P, D], F32, tag=f"acc{i}", name=f"acc{i}") for i in range(n_acc)]
        for i in range(n_acc):
            nc.vector.memset(acc[i][:], 0.0)

        for a in range(3):
            Ka = KS[a]
            off = offs[a]
            nkt = Ka // P  # k-tiles per chunk for this attention
            pieces = [(p0, min(PIECE, Ka - p0)) for p0 in range(0, Ka, PIECE)]
            use_poly = a in poly_attns
            for qc in range(n_chunks):
                q0 = qc * QC
                # ---------------- pass 1: scores + exp + S1 ----------------
                s1_ps = ps_m.tile([1, QC], F32, tag="s1", name="s1")
                wins = []
                jt = 0
                for (p0, plen) in pieces:
                    ktt = [kt_p.tile([P, PIECE], BF16, tag="kt", name="kt") for _ in range(2)]
                    for d in range(2):
                        nc.gpsimd.dma_start(
                            ktt[d][:, 0:plen], kt_d[d][:, off + p0 : off + p0 + plen]
                        )
                    for j in range(plen // P):
                        if jt % WIN == 0:
                            win = strip_p.tile([P, WIN * QC], BF16, tag="win", name="win")
                            wins.append(win)
                        w_off = (jt % WIN) * QC
                        s_ps = ps_s.tile([P, QC], F32, tag="s", name="s")
                        nc.tensor.matmul(
                            s_ps[:],
                            ktt[0][:, j * P : (j + 1) * P],
                            hqt[0][:, q0 : q0 + QC],
                            start=True,
                            stop=False,
                        )
                        nc.tensor.matmul(
                            s_ps[:],
                            ktt[1][:, j * P : (j + 1) * P],
                            hqt[1][:, q0 : q0 + QC],
                            start=False,
                            stop=True,
                        )
                        nc.scalar.activation(
                            win[:, w_off : w_off + QC], s_ps[:], AFT.Exp
                        )
                        nc.tensor.matmul(
                            s1_ps[:],
                            ones_k[:],
                            win[:, w_off : w_off + QC],
                            start=(jt == 0),
                            stop=(jt == nkt - 1),
                        )
                        jt += 1
                # ---------------- mid: r1 and broadcast ----------------
                r1 = small_p.tile([1, QC], F32, tag="r1", name="r1")
                nc.vector.reciprocal(r1[:], s1_ps[:])
                r1_ps = ps_m.tile([P, QC], F32, tag="r1b", name="r1b")
                nc.tensor.matmul(r1_ps[:], ones_r[:], r1[:], start=True, stop=True)
                r1b = small_p.tile([P, QC], BF16, tag="r1bf", name="r1bf")
                nc.scalar.activation(r1b[:], r1_ps[:], AFT.Copy)

                # ---------------- pass 2: normalize, mask, u, PV ----------------
                o_ps = [ps_o.tile([P, D + 1], F32, tag="o", name="o") for _ in range(QC // P)]
                r1_rep = r1b[:][:, None, :].broadcast_to((P, WIN, QC))
                for w, win in enumerate(wins):
                    win3 = win[:].rearrange("p (j q) -> p j q", q=QC)
                    nc.vector.tensor_tensor(win3, win3, r1_rep, ALU.mult)
                    mwin = mwin_p.tile([P, WIN * QC], BF16, tag="m", name="m")
                    m_src = mt_d[
                        off + w * WIN * P : off + (w + 1) * WIN * P, q0 : q0 + QC
                    ].rearrange("(j p) q -> p j q", p=P)
                    nc.sync.dma_start(
                        mwin[:].rearrange("p (j q) -> p j q", q=QC), m_src
                    )
                    nc.vector.tensor_tensor(win[:], win[:], mwin[:], ALU.mult)
                    if use_poly:
                        # u = arg * (1 + arg/2)
                        y = mwin_p.tile([P, WIN * QC], BF16, tag="y", name="y")
                        nc.vector.tensor_scalar(
                            y[:], win[:], 0.5, 1.0, ALU.mult, ALU.add
                        )
                        nc.vector.tensor_tensor(win[:], win[:], y[:], ALU.mult)
                    else:
                        nc.scalar.activation(win[:], win[:], AFT.Exp)
                        nc.vector.tensor_scalar(
                            win[:], win[:], 1.0, None, ALU.subtract
                        )
                    for j in range(WIN):
                        ktile = w * WIN + j
                        vt = vt_p.tile([P, D + 1], BF16, tag="v", name="v")
                        nc.sync.dma_start(
                            vt[:], vp_d[off + ktile * P : off + (ktile + 1) * P, :]
                        )
                        for s in range(QC // P):
                            nc.tensor.matmul(
                                o_ps[s][:],
                                win[:, j * QC + s * P : j * QC + (s + 1) * P],
                                vt[:],
                                start=(ktile == 0),
                                stop=(ktile == nkt - 1),
                            )
                # ---------------- finalize ----------------
                for s in range(QC // P):
                    qi = (q0 + s * P) // P
                    t1 = small_p.tile([P, 1], F32, tag="t1", name="t1")
                    nc.vector.tensor_scalar(
                        t1[:], o_ps[s][:, D : D + 1], 3.0 * Ka, None, ALU.add
                    )
                    r2 = small_p.tile([P, 1], F32, tag="r2", name="r2")
                    nc.vector.reciprocal(r2[:], t1[:])
                    nc.vector.scalar_tensor_tensor(
                        acc[qi][:], o_ps[s][:, 0:D], r2[:], acc[qi][:],
                        ALU.mult, ALU.add,
                    )
                    nc.vector.scalar_tensor_tensor(
                        acc[qi][:], csv[a][:], r2[:], acc[qi][:],
                        ALU.mult, ALU.add,
                    )

        for i in range(n_acc):
            nc.sync.dma_start(out_d[i * P : (i + 1) * P, :], acc[i][:])

    nc.compile()
    return nc


def host_prep(L, H, B_low, H_low, B_high, H_high, n_cores=N_CORES):
    """Build per-core input maps (all layout prep on host)."""
    N, d = H.shape
    Q = N // n_cores
    K_all = np.concatenate([H, H_low, H_high], axis=0)  # [KTOT, D] f32
    KTOT = K_all.shape[0]
    kt = np.ascontiguousarray(K_all.T.astype(NPBF)).reshape(2, P, KTOT)
    vp = np.concatenate(
        [K_all.astype(NPBF), np.full((KTOT, 1), 3.0, NPBF)], axis=1
    )
    vp = np.ascontiguousarray(vp)
    KS = (H.shape[0], H_low.shape[0], H_high.shape[0])
    csv = np.stack(
        [
            np.broadcast_to(
                K_all[o : o + k].sum(axis=0, dtype=np.float64).astype(np.float32),
                (P, d),
            )
            for o, k in zip((0, KS[0], KS[0] + KS[1]), KS)
        ]
    )
    csv = np.ascontiguousarray(csv)

    in_maps = []
    for c in range(n_cores):
        rows = slice(c * Q, (c + 1) * Q)
        hqt = np.ascontiguousarray((H[rows] / 16.0).T.astype(NPBF)).reshape(2, P, Q)
        mt = np.concatenate(
            [
                (L[rows] != 0).T,
                (B_low[:, rows] != 0),
                (B_high[rows] != 0).T,
            ],
            axis=0,
        ).astype(NPBF)
        mt = np.ascontiguousarray(mt)
        in_maps.append({"hqt": hqt, "kt": kt, "mt": mt, "vp": vp, "csv": csv})
    return in_maps, KS, Q


_NC_CACHE = {}


def kernel(L, H, B_low, H_low, B_high, H_high):
    L = np.asarray(L)
    H = np.asarray(H, dtype=np.float32)
    B_low = np.asarray(B_low)
    H_low = np.asarray(H_low, dtype=np.float32)
    B_high = np.asarray(B_high)
    H_high = np.asarray(H_high, dtype=np.float32)

    in_maps, KS, Q = host_prep(L, H, B_low, H_low, B_high, H_high)
    key = (Q, KS)
    if key not in _NC_CACHE:
        _NC_CACHE[key] = build_nc(Q, KS)
    nc = _NC_CACHE[key]
    res = run_bass_kernel_spmd(nc, in_maps, core_ids=list(range(N_CORES)))
    out = np.concatenate(
        [np.asarray(res.results[c]["out"]) for c in range(N_CORES)], axis=0
    )
    return out.astype(np.float32)
